# revision 1
# baseline (speedup 1.0000x reference)
"""Distributed contrastive-loss kernel for Trainium2 (8 NeuronCores).

Strategy (row-sharded, fp8 all-gather of normalized ehr^T):
  - core c owns rows [1024c, 1024c+1024) of both feature matrices
  - phase 1a: normalize local ehr shard -> fp8, PE-transpose to [D, rows],
    AllGather ehr^T (fp8, half the bf16 bytes)
  - phase 1b (overlapped with the AllGather flight): normalize cxr
    (inv-temp folded), fp8 transpose, diag via DVE mul + Pool reduce
  - phase 2: G-block = cn_shard @ en_full^T via fp8 DoubleRow matmuls
    (2x contraction per instr), exp fused with row-sums on ACT
    (accum_out), column partials via PSUM-accumulated ones-matmuls
  - phase 3: AllGather of [colsum(64)|nll1(1)] per-partition partials,
    local sum + logs; core 0's output is read by the host.

Activation-table hygiene: phase 1 uses Square+Rsqrt (one table),
phase 2/3 use Exp then Ln (batched) — ~4 table loads total.
"""

import numpy as np

N = 8192
D = 512
NC = 8
SHARD = N // NC  # 1024
P = 128

_cached = None


def _build(N=N, D=D, NC=NC):
    SHARD = N // NC
    KC = D // P       # 4 contraction chunks of 128
    MC = SHARD // P   # 8 row chunks per core
    _EHRT_BYTES = KC * SHARD          # fp8 bytes per partition (4096)
    _EH_F32 = _EHRT_BYTES // 4        # same region in f32 words (1024)
    _EH2 = P * _EH_F32                # flat f32 size of ehrT region
    _AGW = _EH2                       # AG payload: ehrT only
    _ARW = P * (N // P // NC + 1)     # [P, 65] f32 partial block

    import concourse.bass as bass
    import concourse.tile as tile
    from concourse import bacc, mybir
    from concourse.masks import make_identity

    f32 = mybir.dt.float32
    bf16 = mybir.dt.bfloat16
    fp8 = mybir.dt.float8e4
    AF = mybir.ActivationFunctionType
    ALU = mybir.AluOpType
    X = mybir.AxisListType.X
    DR = mybir.MatmulPerfMode.DoubleRow

    nc = bacc.Bacc("TRN2", debug=False, num_devices=NC)

    cxr_d = nc.dram_tensor("cxr", [SHARD, D], f32, kind="ExternalInput").ap()
    ehr_d = nc.dram_tensor("ehr", [SHARD, D], f32, kind="ExternalInput").ap()
    temp_d = nc.dram_tensor("temp", [1, 1], f32, kind="ExternalInput").ap()
    loss_d = nc.dram_tensor("loss", [1, 1], f32, kind="ExternalOutput").ap()

    groups = [list(range(NC))]
    CW = N // P // NC  # 8 colsum words per partition per core-section; [P,64] total

    with tile.TileContext(nc) as tc:
        from contextlib import ExitStack

        with ExitStack() as ctx:
            singles = ctx.enter_context(tc.tile_pool(name="singles", bufs=1))
            dram = ctx.enter_context(tc.tile_pool(name="dram", bufs=1, space="DRAM"))

            # persistent SBUF tensors
            cnT = singles.tile([P, KC * SHARD], fp8)     # lhsT: col = k*SHARD + i
            ehrT = singles.tile([P, KC * SHARD], fp8)    # local ehr^T (send buf)
            ehrN = singles.tile([P, MC * D], bf16)       # normalized ehr rows
            # per-source-core rhs slabs, split into column halves (h0/h1)
            # so the first AllGather half unblocks phase 2 early
            enTh = [
                [
                    singles.tile([P, KC * 512], fp8, name=f"enT{c}h{h}")
                    for h in range(2)
                ]
                for c in range(NC)
            ]
            diag = singles.tile([P, MC], f32)            # cos_ii / t, local rows
            diag_all = singles.tile([P, N // P], f32)    # [128, 64] all rows
            rowscr = singles.tile([P, MC * NC * 2], f32)  # per (m, n, h) row sums
            identity_bf = singles.tile([P, P], bf16)
            ones_bf = singles.tile([P, P], bf16)
            ones_f32 = singles.tile([P, 1], f32)

            make_identity(nc, identity_bf[:, :])
            nc.vector.memset(ones_bf[:, :], 1.0)
            nc.vector.memset(ones_f32[:, :], 1.0)

            # temperature -> inv_t broadcast on all partitions
            t_sb = singles.tile([P, 1], f32)
            nc.gpsimd.dma_start(out=t_sb[:, :], in_=temp_d.to_broadcast([P, 1]))
            inv_t = singles.tile([P, 1], f32)
            nc.vector.reciprocal(inv_t[:, :], t_sb[:, :])

            # collective DRAM buffers — the big AllGather is split in two
            # halves (ehr rows i<512 / i>=512) to overlap with phase 2
            _HW = _EH2 // 2
            ag_inh = [dram.tile([_HW], f32, name=f"ag_in{h}") for h in range(2)]
            ag_outh = [
                dram.tile([NC, _HW], f32, addr_space="Shared", name=f"ag_out{h}")
                for h in range(2)
            ]

            # ---------------- phase 1a: ehr normalize + transpose ----------
            with ExitStack() as p1:
                natp = p1.enter_context(tc.tile_pool(name="natp", bufs=2))
                smallp = p1.enter_context(tc.tile_pool(name="smallp", bufs=4))
                scrp = p1.enter_context(tc.tile_pool(name="scrp", bufs=2))
                ptp = p1.enter_context(
                    tc.tile_pool(name="ptp", bufs=2, space="PSUM")
                )

                def norm_chunk(xin, xout, fold_t):
                    """normalize 128 rows into fp8 (optionally fold 1/t)."""
                    sq = scrp.tile([P, D], bf16, tag="sq")
                    ssq = smallp.tile([P, 1], f32, tag="ssq")
                    nc.scalar.activation(
                        sq[:, :], xin[:, :], AF.Square, accum_out=ssq[:, :]
                    )
                    nrm = smallp.tile([P, 1], f32, tag="nrm")
                    nc.scalar.activation(nrm[:, :], ssq[:, :], AF.Sqrt)
                    inv = smallp.tile([P, 1], f32, tag="inv")
                    nc.vector.reciprocal(inv[:, :], nrm[:, :])
                    if fold_t:
                        invt = smallp.tile([P, 1], f32, tag="invt")
                        nc.vector.tensor_mul(invt[:, :], inv[:, :], inv_t[:, :])
                        inv = invt
                    nc.vector.tensor_scalar_mul(xout[:, :], xin[:, :], inv[:, :])

                def transpose_chunk(xb, dstT, m):
                    # transpose in bf16 (fp8 PSUM outputs are rejected by
                    # walrus); the PSUM->SBUF copy performs the fp8 cast
                    pt = ptp.tile([P, KC * P], bf16, space="PSUM", tag="pt")
                    for k in range(KC):
                        nc.tensor.transpose(
                            pt[:, k * P : (k + 1) * P],
                            xb[:, k * P : (k + 1) * P],
                            identity_bf[:, :],
                        )
                    dst = dstT[:, :].rearrange("p (k i) -> p k i", k=KC)[
                        :, :, m * P : (m + 1) * P
                    ]
                    nc.vector.tensor_copy(
                        out=dst, in_=pt[:, :].rearrange("p (k i) -> p k i", k=KC)
                    )

                for m in range(MC):
                    rs = slice(m * P, (m + 1) * P)
                    xe = natp.tile([P, D], f32, tag="xe")
                    nc.sync.dma_start(out=xe[:, :], in_=ehr_d[rs, :])
                    xeb = ehrN[:, m * D : (m + 1) * D]
                    norm_chunk(xe, xeb, fold_t=False)
                    transpose_chunk(xeb, ehrT, m)

                # ship ehr^T halves (fp8, bitcast into f32 payload); two
                # AllGathers so the first lands while phase 1b/2 compute
                ehrT_v = ehrT[:, :].rearrange("p (k i) -> p k i", k=KC)
                for h in range(2):
                    nc.sync.dma_start(
                        out=ag_inh[h][0:_HW]
                        .bitcast(fp8)
                        .rearrange("(p k i) -> p k i", p=P, k=KC),
                        in_=ehrT_v[:, :, h * 512 : (h + 1) * 512],
                    )
                    nc.gpsimd.collective_compute(
                        "AllGather",
                        ALU.bypass,
                        replica_groups=groups,
                        ins=[ag_inh[h][:]],
                        outs=[ag_outh[h][:, :]],
                    )

                # ---- phase 1b: cxr side, overlapped with the AG flight ----
                for m in range(MC):
                    rs = slice(m * P, (m + 1) * P)
                    xc = natp.tile([P, D], f32, tag="xc")
                    nc.sync.dma_start(out=xc[:, :], in_=cxr_d[rs, :])
                    xcb = scrp.tile([P, D], bf16, tag="xcb")
                    norm_chunk(xc, xcb, fold_t=True)
                    transpose_chunk(xcb, cnT, m)
                    # diag_m = sum_d xcb*xeb  (1/t already folded into xcb)
                    dscr = scrp.tile([P, D], bf16, tag="dscr")
                    nc.vector.tensor_mul(
                        dscr[:, :], xcb[:, :], ehrN[:, m * D : (m + 1) * D]
                    )
                    nc.vector.reduce_sum(diag[:, m : m + 1], dscr[:, :], axis=X)

            # gather back: per-source-core, per-half fp8 slabs
            for h in range(2):
                for c in range(NC):
                    src = (
                        ag_outh[h][c, 0:_HW]
                        .bitcast(fp8)
                        .rearrange("(p ki) -> p ki", p=P)
                    )
                    nc.sync.dma_start(out=enTh[c][h][:, :], in_=src)

            # local diag -> every core needs every diag for nll2 (phase 3).
            # Pack diag into the same AllGather: reuse ag payload tail.
            # (diag was not ready at AG time on the ehr-first schedule, so
            # gather diag with the second, small AllGather instead.)

            # ------------- phase 2: main similarity block -------------
            stage = singles.tile([P, CW * NC + 1], f32)

            with ExitStack() as p2:
                pmp = p2.enter_context(
                    tc.tile_pool(name="pmp", bufs=2, space="PSUM")
                )
                pcp = p2.enter_context(
                    tc.tile_pool(name="pcp", bufs=1, space="PSUM")
                )
                expp = p2.enter_context(tc.tile_pool(name="expp", bufs=3))

                colT = pcp.tile([P, N // P], f32, space="PSUM")
                lhs_view = cnT[:, :].rearrange("p (k i) -> p k i", k=KC)

                for h in range(2):
                    for n in range(NC):
                        rhs_view = enTh[n][h][:, :].rearrange(
                            "p (k j) -> p k j", k=KC
                        )
                        for m in range(MC):
                            pm = pmp.tile([P, 512], f32, space="PSUM", tag="pm")
                            for k2 in range(KC // 2):
                                nc.tensor.matmul(
                                    pm[:, :],
                                    lhsT=lhs_view[
                                        :, 2 * k2 : 2 * k2 + 2, m * P : (m + 1) * P
                                    ],
                                    rhs=rhs_view[:, 2 * k2 : 2 * k2 + 2, :],
                                    start=(k2 == 0),
                                    stop=(k2 == KC // 2 - 1),
                                    perf_mode=DR,
                                )
                            et = expp.tile([P, 512], bf16, tag="et")
                            ri = (m * NC + n) * 2 + h
                            nc.scalar.activation(
                                et[:, :],
                                pm[:, :],
                                AF.Exp,
                                accum_out=rowscr[:, ri : ri + 1],
                            )
                            # column partials, accumulated over m in PSUM:
                            # colT[:, n*8+h*4+t] += et[:, 128t:+128]^T @ ones
                            for t in range(4):
                                ci = n * MC + h * 4 + t
                                nc.tensor.matmul(
                                    colT[:, ci : ci + 1],
                                    lhsT=et[:, t * P : (t + 1) * P],
                                    rhs=ones_bf[:, 0:1],
                                    start=(m == 0),
                                    stop=(m == MC - 1),
                                    skip_group_check=True,
                                )

                nc.vector.tensor_copy(out=stage[:, 0 : N // P], in_=colT[:, :])

            # --------- phase 3: local nll1, gather partials, finish ---------
            rowsum = singles.tile([P, MC], f32)
            for m in range(MC):
                nc.vector.reduce_sum(
                    rowsum[:, m : m + 1],
                    rowscr[:, m * NC * 2 : (m + 1) * NC * 2],
                    axis=X,
                )
            expd = singles.tile([P, MC], f32)
            nc.scalar.activation(expd[:, :], diag[:, :], AF.Exp)
            rs_ns = singles.tile([P, MC], f32)
            nc.vector.tensor_sub(rs_ns[:, :], rowsum[:, :], expd[:, :])
            lse1 = singles.tile([P, MC], f32)
            nc.scalar.activation(lse1[:, :], rs_ns[:, :], AF.Ln)
            nll1 = singles.tile([P, MC], f32)
            nc.vector.tensor_sub(nll1[:, :], diag[:, :], lse1[:, :])
            nc.vector.reduce_sum(
                stage[:, N // P : N // P + 1], nll1[:, :], axis=X
            )

            # second AllGather: [colsum(64) | nll1(1)] f32 + local diag(8)
            ar_stage = singles.tile([P, CW * NC + 1 + MC], f32)
            nc.vector.tensor_copy(
                out=ar_stage[:, 0 : CW * NC + 1], in_=stage[:, :]
            )
            nc.vector.tensor_copy(
                out=ar_stage[:, CW * NC + 1 :], in_=diag[:, :]
            )
            _ARW2 = P * (CW * NC + 1 + MC)
            ar_in2 = dram.tile([_ARW2], f32, name="ar_in2")
            ar_out2 = dram.tile([NC, _ARW2], f32, addr_space="Shared", name="ar_out2")
            nc.sync.dma_start(
                out=ar_in2[0:_ARW2].rearrange("(p w) -> p w", p=P),
                in_=ar_stage[:, :],
            )
            nc.gpsimd.collective_compute(
                "AllGather",
                ALU.bypass,
                replica_groups=groups,
                ins=[ar_in2[:]],
                outs=[ar_out2[:, :]],
            )

            WB = CW * NC + 1 + MC
            arb = singles.tile([P, NC * WB], f32)
            for c in range(NC):
                nc.sync.dma_start(
                    out=arb[:, c * WB : (c + 1) * WB],
                    in_=ar_out2[c, :].rearrange("(p w) -> p w", p=P),
                )
            # sum colsum+nll1 partials across cores; collect diag_all
            summed = singles.tile([P, CW * NC + 1], f32)
            nc.vector.tensor_copy(out=summed[:, :], in_=arb[:, 0 : CW * NC + 1])
            for c in range(1, NC):
                nc.vector.tensor_add(
                    summed[:, :],
                    summed[:, :],
                    arb[:, c * WB : c * WB + CW * NC + 1],
                )
            for c in range(NC):
                nc.vector.tensor_copy(
                    out=diag_all[:, c * MC : (c + 1) * MC],
                    in_=arb[:, c * WB + CW * NC + 1 : (c + 1) * WB],
                )

            expd_all = singles.tile([P, N // P], f32)
            nc.scalar.activation(expd_all[:, :], diag_all[:, :], AF.Exp)
            cs_ns = singles.tile([P, N // P], f32)
            nc.vector.tensor_sub(
                cs_ns[:, :], summed[:, 0 : N // P], expd_all[:, :]
            )
            lse2 = singles.tile([P, N // P], f32)
            nc.scalar.activation(lse2[:, :], cs_ns[:, :], AF.Ln)
            nll2 = singles.tile([P, N // P], f32)
            nc.vector.tensor_sub(nll2[:, :], diag_all[:, :], lse2[:, :])
            t2 = singles.tile([P, 1], f32)
            nc.vector.reduce_sum(t2[:, :], nll2[:, :], axis=X)
            tfin = singles.tile([P, 1], f32)
            nc.vector.tensor_add(
                tfin[:, :], t2[:, :], summed[:, N // P : N // P + 1]
            )

            with tc.tile_pool(name="psfin", bufs=1, space="PSUM") as psfin:
                s2ps = psfin.tile([1, 1], f32, space="PSUM")
                nc.tensor.matmul(
                    s2ps[:, :],
                    lhsT=ones_f32[:, 0:1],
                    rhs=tfin[:, :],
                    start=True,
                    stop=True,
                )
                tot = singles.tile([1, 1], f32)
                nc.vector.tensor_copy(out=tot[:, :], in_=s2ps[:, :])

            out_sb = singles.tile([1, 1], f32)
            nc.vector.tensor_scalar_mul(out_sb[:, :], tot[:, :], -1.0 / N)
            nc.sync.dma_start(out=loss_d[:, :], in_=out_sb[:, :])

    nc.compile()
    return nc


def _get_nc():
    global _cached
    if _cached is None:
        _cached = _build()
    return _cached


def _make_in_maps(cxr_feats, ehr_feats, temperature):
    cxr = np.ascontiguousarray(np.asarray(cxr_feats, dtype=np.float32))
    ehr = np.ascontiguousarray(np.asarray(ehr_feats, dtype=np.float32))
    t = np.asarray(temperature, dtype=np.float32).reshape(1, 1)
    in_maps = []
    for c in range(NC):
        sl = slice(c * SHARD, (c + 1) * SHARD)
        in_maps.append(
            {
                "cxr": np.ascontiguousarray(cxr[sl]),
                "ehr": np.ascontiguousarray(ehr[sl]),
                "temp": t,
            }
        )
    return in_maps


def run(cxr_feats, ehr_feats, temperature, trace=False):
    """Returns (loss_scalar, BassKernelResults)."""
    from concourse import bass_utils

    nc = _get_nc()
    in_maps = _make_in_maps(cxr_feats, ehr_feats, temperature)
    res = bass_utils.run_bass_kernel_spmd(
        nc, in_maps, core_ids=list(range(NC)), trace=trace
    )
    loss = np.float32(np.asarray(res.results[0]["loss"]).reshape(-1)[0])
    return np.asarray(loss, dtype=np.float32).reshape(()), res


def kernel(cxr_feats, ehr_feats, temperature):
    loss, _ = run(cxr_feats, ehr_feats, temperature, trace=False)
    return loss



# revision 10
# speedup vs baseline: 1.0820x; 1.0820x over previous
"""Distributed contrastive-loss kernel for Trainium2 (8 NeuronCores).

Strategy (row-sharded, fp8 all-gather of normalized ehr^T):
  - core c owns rows [1024c, 1024c+1024) of both feature matrices
  - phase 1: normalize ehr shard (Square+accum -> inv = Exp(-0.5*Ln(ssq)),
    single activation-table set), fp8 PE-transpose, ship ehr^T in two
    AllGather halves (m 0-3 / 4-7) so the first lands early; then the
    cxr side (1/t folded via Exp bias = -Ln(t)) + diag; small diag AG.
  - phase 2: h0 then h1 sub-phases; per (m,h): 4+4 (h0) or 4+3+1 (h1)
    slab batches -> [128, 2048] PSUM tiles, DR fp8 matmuls, one Exp
    ACTIVATE per batch (accum_out = per-m rowsum slots), DVE bf16
    accumulation over m into ET; column sums via ET-as-weights
    [128,1]-out matmuls into a dedicated colT bank, h0's fired during
    the h1 sub-phase.
  - phase 3: AllGather [colsum(64) | nll1(1)] partials, local sum +
    Ln + reduction; core 0's output is read by the host.

Activation-table hygiene: build-time patch of the table-set map makes
Square/Ln/Exp all resolve to natural_log_exp_and_others -> ONE load.
"""

import numpy as np

N = 8192
D = 512
NC = 8
SHARD = N // NC  # 1024
P = 128

_cached = None


def _patch_act_tables():
    """Make the act-table insertion pass map Square/Ln/Exp to the one
    set that holds all three (natural_log_exp_and_others), so the whole
    kernel needs a single ACT_TABLE_LOAD instead of thrashing between
    exp_and_others and natural_log. Set order/IDs are preserved; only
    membership is filtered, so the emitted set id stays valid."""
    import concourse.hw_specs as hw_specs
    import concourse.bacc as bacc_mod
    import concourse.mybir as mybir

    if getattr(bacc_mod.get_activation_tables, "_contrastive_patch", False):
        return
    _orig = hw_specs.get_activation_tables
    AF = mybir.ActivationFunctionType

    def patched(arch):
        tables = _orig(arch)
        for name, funcs in tables.items():
            if name != "natural_log_exp_and_others":
                funcs.discard(AF.Exp)
                funcs.discard(AF.Ln)
                funcs.discard(AF.Square)
        return tables

    patched._contrastive_patch = True
    bacc_mod.get_activation_tables = patched


def _build(N=N, D=D, NC=NC):
    SHARD = N // NC
    KC = D // P       # 4 contraction chunks of 128
    MC = SHARD // P   # 8 row chunks per core
    HW = P * KC * 512 // 4   # f32 words per AG half payload (65536)

    _patch_act_tables()

    import concourse.bass as bass
    import concourse.tile as tile
    from concourse import bacc, mybir
    from concourse.masks import make_identity

    f32 = mybir.dt.float32
    bf16 = mybir.dt.bfloat16
    fp8 = mybir.dt.float8e4
    AF = mybir.ActivationFunctionType
    ALU = mybir.AluOpType
    X = mybir.AxisListType.X
    DR = mybir.MatmulPerfMode.DoubleRow

    nc = bacc.Bacc("TRN2", debug=False, num_devices=NC)

    cxr_d = nc.dram_tensor("cxr", [SHARD, D], f32, kind="ExternalInput").ap()
    ehr_d = nc.dram_tensor("ehr", [SHARD, D], f32, kind="ExternalInput").ap()
    temp_d = nc.dram_tensor("temp", [1, 1], f32, kind="ExternalInput").ap()
    loss_d = nc.dram_tensor("loss", [1, 1], f32, kind="ExternalOutput").ap()

    groups = [list(range(NC))]
    NB = N // P          # 64 column-sum slots
    RS = 65              # AG payload width: 64 colsum + 1 nll1

    with tile.TileContext(nc) as tc:
        from contextlib import ExitStack

        with ExitStack() as ctx:
            singles = ctx.enter_context(tc.tile_pool(name="singles", bufs=1))
            dram = ctx.enter_context(tc.tile_pool(name="dram", bufs=1, space="DRAM"))

            # persistent SBUF tensors
            enT = singles.tile([P, KC * N], fp8)        # all-source rhs slabs
            cnT = singles.tile([P, KC * SHARD], fp8)    # lhsT weights
            ehrT = singles.tile([P, KC * SHARD], fp8)   # local ehr^T (AG send)
            ehrN = singles.tile([P, MC * D], bf16)      # normalized ehr rows
            ET = singles.tile([P, N], bf16)             # m-accumulated exp
            diag = singles.tile([P, MC], f32)
            diag_all = singles.tile([P, NB], f32)
            expd = singles.tile([P, MC], f32)
            expd_all = singles.tile([P, NB], f32)
            rowacc = singles.tile([P, MC * 5], f32)     # per-m rowsum slots
            colstage = singles.tile([P, RS], f32)
            identity_bf = singles.tile([P, P], bf16)
            ones_bf = singles.tile([P, 1], bf16)
            ones_f32 = singles.tile([P, 1], f32)

            make_identity(nc, identity_bf[:, :])
            nc.vector.memset(ones_bf[:, :], 1.0)
            nc.vector.memset(ones_f32[:, :], 1.0)

            # temperature -> -ln(t) broadcast (Exp bias for cxr normalize)
            t_sb = singles.tile([P, 1], f32)
            nc.gpsimd.dma_start(out=t_sb[:, :], in_=temp_d.to_broadcast([P, 1]))
            lnt = singles.tile([P, 1], f32)
            nc.scalar.activation(lnt[:, :], t_sb[:, :], AF.Ln)
            neg_lnt = singles.tile([P, 1], f32)
            nc.vector.tensor_scalar_mul(neg_lnt[:, :], lnt[:, :], -1.0)

            # collective DRAM buffers
            ag_inh = [dram.tile([HW], f32, name=f"ag_in{h}") for h in range(2)]
            ag_outh = [
                dram.tile([NC, HW], f32, addr_space="Shared", name=f"ag_out{h}")
                for h in range(2)
            ]
            agd_in = dram.tile([P * MC], f32, name="agd_in")
            agd_out = dram.tile([NC, P * MC], f32, addr_space="Shared", name="agd_out")
            agr_in = dram.tile([P * RS], f32, name="agr_in")
            agr_out = dram.tile([NC, P * RS], f32, addr_space="Shared", name="agr_out")

            ehrT_v = ehrT[:, :].rearrange("p (k i) -> p k i", k=KC)
            cnT_v = cnT[:, :].rearrange("p (k i) -> p k i", k=KC)
            enT_v = enT[:, :].rearrange("p (k j) -> p k j", k=KC)

            # ---------------- phase 1: normalize + transpose ----------
            with ExitStack() as p1:
                natp = p1.enter_context(tc.tile_pool(name="natp", bufs=5))
                scrp = p1.enter_context(tc.tile_pool(name="scrp", bufs=2))
                smallp = p1.enter_context(tc.tile_pool(name="smallp", bufs=2))
                ptp = p1.enter_context(
                    tc.tile_pool(name="ptp", bufs=2, space="PSUM")
                )

                ssq_e = singles.tile([P, MC], f32)
                ssq_c = singles.tile([P, MC], f32)
                inv_e = singles.tile([P, MC], f32)
                inv_c = singles.tile([P, MC], f32)

                def transpose_chunk(xb, dstT, m):
                    # bf16 PE transpose; PSUM->SBUF copy casts to fp8
                    pt = ptp.tile([P, KC * P], bf16, space="PSUM", tag="pt")
                    for k in range(KC):
                        nc.tensor.transpose(
                            pt[:, k * P : (k + 1) * P],
                            xb[:, k * P : (k + 1) * P],
                            identity_bf[:, :],
                        )
                    dst = dstT.rearrange("p (k i) -> p k i", k=KC)[
                        :, :, m * P : (m + 1) * P
                    ]
                    nc.vector.tensor_copy(
                        out=dst, in_=pt[:, :].rearrange("p (k i) -> p k i", k=KC)
                    )

                # --- ehr, in two halves; AG each half as soon as ready ---
                xe_tiles = {}
                for half in range(2):
                    for m in range(half * 4, half * 4 + 4):
                        rs = slice(m * P, (m + 1) * P)
                        xe = natp.tile([P, D], f32, tag="xe")
                        nc.sync.dma_start(out=xe[:, :], in_=ehr_d[rs, :])
                        sq = scrp.tile([P, D], bf16, tag="sq")
                        nc.scalar.activation(
                            sq[:, :], xe[:, :], AF.Square,
                            accum_out=ssq_e[:, m : m + 1],
                        )
                        xe_tiles[m] = xe
                    hs = slice(half * 4, half * 4 + 4)
                    lns = smallp.tile([P, 4], f32, tag="lns")
                    nc.scalar.activation(lns[:, :], ssq_e[:, hs], AF.Ln)
                    nc.scalar.activation(
                        inv_e[:, hs], lns[:, :], AF.Exp, scale=-0.5
                    )
                    for m in range(half * 4, half * 4 + 4):
                        xnb = ehrN[:, m * D : (m + 1) * D]
                        nc.vector.tensor_scalar_mul(
                            xnb, xe_tiles[m][:, :], inv_e[:, m : m + 1]
                        )
                        transpose_chunk(xnb, ehrT[:, :], m)
                    # ship this half (fp8 bitcast into f32 payload)
                    nc.sync.dma_start(
                        out=ag_inh[half][0:HW]
                        .bitcast(fp8)
                        .rearrange("(p k i) -> p k i", p=P, k=KC),
                        in_=ehrT_v[:, :, half * 512 : (half + 1) * 512],
                    )
                    nc.gpsimd.collective_compute(
                        "AllGather",
                        ALU.bypass,
                        replica_groups=groups,
                        ins=[ag_inh[half][:]],
                        outs=[ag_outh[half][:, :]],
                    )

                # --- cxr side (overlaps the AG flight) ---
                xc_tiles = {}
                for half in range(2):
                    for m in range(half * 4, half * 4 + 4):
                        rs = slice(m * P, (m + 1) * P)
                        xc = natp.tile([P, D], f32, tag="xc")
                        nc.sync.dma_start(out=xc[:, :], in_=cxr_d[rs, :])
                        sq = scrp.tile([P, D], bf16, tag="sq")
                        nc.scalar.activation(
                            sq[:, :], xc[:, :], AF.Square,
                            accum_out=ssq_c[:, m : m + 1],
                        )
                        xc_tiles[m] = xc
                    hs = slice(half * 4, half * 4 + 4)
                    lns = smallp.tile([P, 4], f32, tag="lns")
                    nc.scalar.activation(lns[:, :], ssq_c[:, hs], AF.Ln)
                    # inv_c = exp(-0.5*ln(ssq) - ln t) = 1/(t*norm)
                    nc.scalar.activation(
                        inv_c[:, hs], lns[:, :], AF.Exp,
                        scale=-0.5, bias=neg_lnt[:, :],
                    )
                    for m in range(half * 4, half * 4 + 4):
                        cnb = scrp.tile([P, D], bf16, tag="cnb")
                        nc.vector.tensor_scalar_mul(
                            cnb[:, :], xc_tiles[m][:, :], inv_c[:, m : m + 1]
                        )
                        transpose_chunk(cnb, cnT[:, :], m)
                        # diag_m = sum_d cnb*ehrN_m (1/t folded into cnb)
                        dscr = scrp.tile([P, D], bf16, tag="dscr")
                        nc.vector.tensor_mul(
                            dscr[:, :], cnb[:, :], ehrN[:, m * D : (m + 1) * D]
                        )
                        nc.vector.reduce_sum(
                            diag[:, m : m + 1], dscr[:, :], axis=X
                        )

                # diag AllGather (small, lands mid-phase-2)
                nc.sync.dma_start(
                    out=agd_in[0 : P * MC].rearrange("(p w) -> p w", p=P),
                    in_=diag[:, :],
                )
                nc.gpsimd.collective_compute(
                    "AllGather",
                    ALU.bypass,
                    replica_groups=groups,
                    ins=[agd_in[:]],
                    outs=[agd_out[:, :]],
                )
                nc.scalar.activation(expd[:, :], diag[:, :], AF.Exp)

            # gather back: per-source fp8 slabs into the contiguous rhs
            for h in range(2):
                for n in range(NC):
                    src = (
                        ag_outh[h][n, 0:HW]
                        .bitcast(fp8)
                        .rearrange("(p k i) -> p k i", p=P, k=KC)
                    )
                    nc.sync.dma_start(
                        out=enT_v[:, :, n * SHARD + h * 512 : n * SHARD + (h + 1) * 512],
                        in_=src,
                    )
            for n in range(NC):
                nc.sync.dma_start(
                    out=diag_all[:, n * MC : (n + 1) * MC],
                    in_=agd_out[n, :].rearrange("(p w) -> p w", p=P),
                )
            nc.scalar.activation(expd_all[:, :], diag_all[:, :], AF.Exp)

            # ------------- phase 2: main similarity block -------------
            # ET column layout: j = n*1024 + h*512 + jj
            ET_hv = ET[:, :].rearrange("p (n h j) -> p n h j", h=2, j=512)

            def mm_batch(pm, m, slabs):
                """DR matmuls for a batch of (n, h) slabs into pm."""
                for s, (n, h) in enumerate(slabs):
                    j0 = n * SHARD + h * 512
                    for k2 in range(KC // 2):
                        nc.tensor.matmul(
                            pm[:, s * 512 : (s + 1) * 512],
                            lhsT=cnT_v[:, 2 * k2 : 2 * k2 + 2, m * P : (m + 1) * P],
                            rhs=enT_v[:, 2 * k2 : 2 * k2 + 2, j0 : j0 + 512],
                            start=(k2 == 0),
                            stop=(k2 == KC // 2 - 1),
                            perf_mode=DR,
                        )

            def et_accumulate(et, m, slabs):
                """DVE: ET[slab cols] (+)= et. Each batch is same-h,
                contiguous-n, so one strided view covers it."""
                h = slabs[0][1]
                n0 = slabs[0][0]
                ln = len(slabs)
                assert all(hh == h for _, hh in slabs)
                assert [n for n, _ in slabs] == list(range(n0, n0 + ln))
                dst = ET_hv[:, n0 : n0 + ln, h : h + 1, :]
                src = et[:, 0 : ln * 512].rearrange(
                    "p (r o j) -> p r o j", o=1, j=512
                )
                if m == 0:
                    nc.vector.tensor_copy(out=dst, in_=src)
                else:
                    nc.vector.tensor_add(dst, dst, src)

            def colsum_mms(colT, slabs):
                """ET-as-weights column sums: [128,1] out per 128-col chunk."""
                for n, h in slabs:
                    j0 = n * SHARD + h * 512
                    for t in range(4):
                        ci = j0 // P + t
                        nc.tensor.matmul(
                            colT[:, ci : ci + 1],
                            lhsT=ET[:, j0 + t * P : j0 + (t + 1) * P],
                            rhs=ones_bf[:, 0:1],
                            start=True,
                            stop=True,
                            skip_group_check=True,
                        )

            # --- h0 sub-phase: batches (4, 4), full 8 banks ---
            with ExitStack() as p2a:
                pmp = p2a.enter_context(tc.tile_pool(name="pmp", bufs=1, space="PSUM"))
                etp = p2a.enter_context(tc.tile_pool(name="etp", bufs=2))
                for m in range(MC):
                    for bi, ns in enumerate((range(0, 4), range(4, 8))):
                        slabs = [(n, 0) for n in ns]
                        pm = pmp.tile(
                            [P, 2048], f32, space="PSUM", tag=f"pm{bi}"
                        )
                        mm_batch(pm, m, slabs)
                        et = etp.tile([P, 2048], bf16, tag=f"et{bi}")
                        nc.scalar.activation(
                            et[:, :], pm[:, :], AF.Exp,
                            accum_out=rowacc[:, m * 5 + bi : m * 5 + bi + 1],
                        )
                        et_accumulate(et, m, slabs)

            # --- h1 sub-phase: batches (4, 3, 1) + colT bank ---
            with ExitStack() as p2b:
                pmp = p2b.enter_context(tc.tile_pool(name="pmp2", bufs=1, space="PSUM"))
                etp = p2b.enter_context(tc.tile_pool(name="etp2", bufs=2))
                colT = pmp.tile([P, NB], f32, space="PSUM", tag="colT")
                h1_batches = [
                    ("pmA", [(n, 1) for n in range(0, 4)]),
                    ("pmB", [(n, 1) for n in range(4, 7)]),
                    ("pmA", [(7, 1)]),
                ]
                fired_h0 = False
                for m in range(MC):
                    for bi, (tag, slabs) in enumerate(h1_batches):
                        w = 512 * len(slabs)
                        pm = pmp.tile([P, 2048 if tag == "pmA" else 1536],
                                      f32, space="PSUM", tag=tag)
                        mm_batch(pm, m, slabs)
                        et = etp.tile([P, w], bf16, tag=f"e{tag}{bi}")
                        nc.scalar.activation(
                            et[:, :], pm[:, 0:w], AF.Exp,
                            accum_out=rowacc[:, m * 5 + 2 + bi : m * 5 + 3 + bi],
                        )
                        et_accumulate(et, m, slabs)
                    if not fired_h0:
                        # h0 columns final: their colsum matmuls overlap h1
                        colsum_mms(colT, [(n, 0) for n in range(NC)])
                        fired_h0 = True
                # h1 columns final per batch at m == MC-1
                colsum_mms(colT, [(n, 1) for n in range(NC)])

                # rowsums: reduce the 6 slots per m, then nll1 partial
                rowsum = singles.tile([P, MC], f32)
                nc.vector.tensor_reduce(
                    rowsum[:, :],
                    rowacc[:, 0 : MC * 5].rearrange("p (m s) -> p m s", s=5),
                    axis=X,
                    op=ALU.add,
                )
                rs_ns = singles.tile([P, MC], f32)
                nc.vector.tensor_sub(rs_ns[:, :], rowsum[:, :], expd[:, :])
                lse1 = singles.tile([P, MC], f32)
                nc.scalar.activation(lse1[:, :], rs_ns[:, :], AF.Ln)
                nll1 = singles.tile([P, MC], f32)
                nc.vector.tensor_sub(nll1[:, :], diag[:, :], lse1[:, :])
                nc.vector.reduce_sum(
                    colstage[:, NB : NB + 1], nll1[:, :], axis=X
                )
                nc.vector.tensor_copy(
                    out=colstage[:, 0:NB], in_=colT[:, :]
                )

            # --------- phase 3: AG partials, finish ---------
            nc.sync.dma_start(
                out=agr_in[0 : P * RS].rearrange("(p w) -> p w", p=P),
                in_=colstage[:, :],
            )
            nc.gpsimd.collective_compute(
                "AllGather",
                ALU.bypass,
                replica_groups=groups,
                ins=[agr_in[:]],
                outs=[agr_out[:, :]],
            )
            arb = singles.tile([P, NC * RS], f32)
            for c in range(NC):
                nc.sync.dma_start(
                    out=arb[:, c * RS : (c + 1) * RS],
                    in_=agr_out[c, :].rearrange("(p w) -> p w", p=P),
                )
            summed = singles.tile([P, RS], f32)
            nc.vector.tensor_copy(out=summed[:, :], in_=arb[:, 0:RS])
            for c in range(1, NC):
                nc.vector.tensor_add(
                    summed[:, :], summed[:, :], arb[:, c * RS : (c + 1) * RS]
                )
            cs_ns = singles.tile([P, NB], f32)
            nc.vector.tensor_sub(cs_ns[:, :], summed[:, 0:NB], expd_all[:, :])
            lse2 = singles.tile([P, NB], f32)
            nc.scalar.activation(lse2[:, :], cs_ns[:, :], AF.Ln)
            nll2 = singles.tile([P, NB], f32)
            nc.vector.tensor_sub(nll2[:, :], diag_all[:, :], lse2[:, :])
            t2 = singles.tile([P, 1], f32)
            nc.vector.reduce_sum(t2[:, :], nll2[:, :], axis=X)
            tfin = singles.tile([P, 1], f32)
            nc.vector.tensor_add(
                tfin[:, :], t2[:, :], summed[:, NB : NB + 1]
            )

            with tc.tile_pool(name="psfin", bufs=1, space="PSUM") as psfin:
                s2ps = psfin.tile([1, 1], f32, space="PSUM")
                nc.tensor.matmul(
                    s2ps[:, :],
                    lhsT=ones_f32[:, 0:1],
                    rhs=tfin[:, :],
                    start=True,
                    stop=True,
                )
                tot = singles.tile([1, 1], f32)
                nc.vector.tensor_copy(out=tot[:, :], in_=s2ps[:, :])

            out_sb = singles.tile([1, 1], f32)
            nc.vector.tensor_scalar_mul(out_sb[:, :], tot[:, :], -1.0 / N)
            nc.sync.dma_start(out=loss_d[:, :], in_=out_sb[:, :])

    nc.compile()
    return nc


def _get_nc():
    global _cached
    if _cached is None:
        _cached = _build()
    return _cached


def _make_in_maps(cxr_feats, ehr_feats, temperature):
    cxr = np.ascontiguousarray(np.asarray(cxr_feats, dtype=np.float32))
    ehr = np.ascontiguousarray(np.asarray(ehr_feats, dtype=np.float32))
    t = np.asarray(temperature, dtype=np.float32).reshape(1, 1)
    in_maps = []
    for c in range(NC):
        sl = slice(c * SHARD, (c + 1) * SHARD)
        in_maps.append(
            {
                "cxr": np.ascontiguousarray(cxr[sl]),
                "ehr": np.ascontiguousarray(ehr[sl]),
                "temp": t,
            }
        )
    return in_maps


def run(cxr_feats, ehr_feats, temperature, trace=False):
    """Returns (loss_scalar, BassKernelResults)."""
    from concourse import bass_utils

    nc = _get_nc()
    in_maps = _make_in_maps(cxr_feats, ehr_feats, temperature)
    res = bass_utils.run_bass_kernel_spmd(
        nc, in_maps, core_ids=list(range(NC)), trace=trace
    )
    loss = np.float32(np.asarray(res.results[0]["loss"]).reshape(-1)[0])
    return np.asarray(loss, dtype=np.float32).reshape(()), res


def kernel(cxr_feats, ehr_feats, temperature):
    loss, _ = run(cxr_feats, ehr_feats, temperature, trace=False)
    return loss
